# revision 8
# baseline (speedup 1.0000x reference)
"""IntraAttention Trainium2 kernel, 8-core SPMD.

Reference computation (N=4096 rows, d=1024):
    Q = X @ Wq.T + bq ; K = X @ Wk.T + bk ; V = X @ Wv.T + bv
    alpha = softmax(Q @ K.T / sqrt(d), axis=1)
    V_ = alpha @ V
    x = concat([V_, Q], axis=1)              # [N, 2d]
    x1 = x @ Wl.T + bl                        # [N, d]
    h = x @ Wa.T + ba                         # [N, 2d]
    out = x1 * (h[:, :d] * sigmoid(h[:, d:]))

Sharding: rows of X are sharded across 8 cores (512 rows each). Q stays
local; K and V shards are all-gathered (K as [d, rows] blocks, V as
[rows, d] blocks) in two pipelined chunks each, so each core runs its
512xN attention + GLU chain locally while the gathers fly. The Q-only
halves of the x1/h projections run while the first gather completes.

On-chip everything is computed transposed ([feature, row] layout) so all
matmul contractions run along the partition axis with N=512 moving free
dim. Matmul operands are fp16 (fp22 multiply, fp32 accumulate in PSUM);
biases/normalization/final multiply run in fp32.
"""

import numpy as np

import concourse.bass as bass
import concourse.bacc as bacc
import concourse.tile as tile
import concourse.bass_utils as bass_utils
from concourse import mybir

P = 128            # partitions
D = 1024           # model dim
N = 4096           # rows
NCORES = 8
R = N // NCORES    # rows per core = 512
HR = R // 2        # half of the local rows = 256
DC = D // P        # d chunks = 8
NK = N // P        # key tiles = 32
TD = 2 * D         # 2048
TDC = TD // P      # 16
HK = 4             # k-chunks of the g0 h-projection computed early (Q part)

F32 = mybir.dt.float32
F16 = mybir.dt.float16

RG = [list(range(NCORES))]

# key-tile visit order: (ss-major) so the first 16 tiles only need chunk 0
# of the K/V allgathers. kt_global = rr*4 + ss; softmax/attention are
# permutation-invariant over keys as long as exp tile i pairs with V rows
# of the same key tile.
KT_ORDER = [(rr, ss) for ss in range(4) for rr in range(NCORES)]


def build_nc():
    nc = bacc.Bacc(
        "TRN2",
        target_bir_lowering=False,
        debug=False,
        num_devices=NCORES,
    )

    # ---- per-core I/O ----
    xt = nc.dram_tensor("xt", [D, R], F16, kind="ExternalInput")      # X_c.T
    wqt = nc.dram_tensor("wqt", [D, D], F16, kind="ExternalInput")    # Wq.T
    wkt = nc.dram_tensor("wkt", [D, D], F16, kind="ExternalInput")    # Wk.T
    wvt = nc.dram_tensor("wvt", [D, D], F16, kind="ExternalInput")    # Wv.T
    wlt = nc.dram_tensor("wlt", [TD, D], F16, kind="ExternalInput")   # Wl.T
    wat = nc.dram_tensor("wat", [TD, TD], F16, kind="ExternalInput")  # Wa.T
    bq = nc.dram_tensor("bq", [P, DC], F32, kind="ExternalInput")
    bk = nc.dram_tensor("bk", [P, DC], F32, kind="ExternalInput")
    bvb = nc.dram_tensor("bvb", [P, D], F32, kind="ExternalInput")    # bv bcast
    bl = nc.dram_tensor("bl", [P, DC], F32, kind="ExternalInput")
    ba = nc.dram_tensor("ba", [P, TDC], F32, kind="ExternalInput")
    out = nc.dram_tensor("out", [D, R], F32, kind="ExternalOutput")   # out_c.T

    # ---- collective buffers ----
    ktc_d = [nc.dram_tensor(f"ktc_d{h}", [D, HR], F16) for h in range(2)]
    vc_d = [nc.dram_tensor(f"vc_d{h}", [HR, D], F16) for h in range(2)]
    ag_k = [nc.dram_tensor(f"ag_k{h}", [NCORES * D, HR], F16, addr_space="Shared")
            for h in range(2)]
    ag_v = [nc.dram_tensor(f"ag_v{h}", [NCORES * HR, D], F16, addr_space="Shared")
            for h in range(2)]

    with tile.TileContext(nc) as tc:
        with (
            tc.tile_pool(name="cpool", bufs=1) as cpool,
            tc.tile_pool(name="wpool", bufs=10) as wpool,
            tc.tile_pool(name="vlpool", bufs=4) as vlpool,
            tc.tile_pool(name="pspool", bufs=8, space="PSUM") as pspool,
        ):
            # constants (scalar-engine DMA queue; sync queue kept for bulk)
            bq_t = cpool.tile([P, DC], F32, name="bq_t")
            bk_t = cpool.tile([P, DC], F32, name="bk_t")
            bl_t = cpool.tile([P, DC], F32, name="bl_t")
            ba_t = cpool.tile([P, TDC], F32, name="ba_t")
            bvb_t = cpool.tile([P, D], F32, name="bvb_t")
            nc.scalar.dma_start(bq_t, bq[:, :])
            nc.scalar.dma_start(bk_t, bk[:, :])
            nc.scalar.dma_start(bl_t, bl[:, :])
            nc.scalar.dma_start(ba_t, ba[:, :])
            nc.scalar.dma_start(bvb_t, bvb[:, :])
            ones_t = cpool.tile([P, 1], F16, name="ones_t")
            nc.vector.memset(ones_t, 1.0)
            ones_row = cpool.tile([1, P], F32, name="ones_row")
            nc.vector.memset(ones_row, 1.0)

            with tc.tile_pool(name="qpool", bufs=1) as qpool, \
                 tc.tile_pool(name="vtpool", bufs=1) as vtpool, \
                 tc.tile_pool(name="qfpool", bufs=1) as qfpool:

                # ============ QKV projections + allgathers ============
                with tc.tile_pool(name="xpool", bufs=1) as xpool, \
                     tc.tile_pool(name="stpool", bufs=4) as stpool:
                    xt_t = [xpool.tile([P, R], F16, name=f"xt{k}") for k in range(DC)]

                    # --- K_c.T = Wk @ X_c.T + bk ---  (first: feeds AG(K))
                    kt_ps = [pspool.tile([P, R], F32, name=f"ktps{m}", tag="ps")
                             for m in range(DC)]
                    for k in range(DC):
                        nc.sync.dma_start(xt_t[k], xt[k * P:(k + 1) * P, :])
                        wk_t = wpool.tile([P, D], F16, name="wk_t", tag="w")
                        nc.sync.dma_start(wk_t, wkt[k * P:(k + 1) * P, :])
                        for m in range(DC):
                            nc.tensor.matmul(
                                kt_ps[m], wk_t[:, m * P:(m + 1) * P], xt_t[k],
                                start=(k == 0), stop=(k == DC - 1))
                    for m in range(DC):
                        st = stpool.tile([P, R], F16, name="st_k", tag="st")
                        nc.vector.tensor_scalar_add(st, kt_ps[m], bk_t[:, m:m + 1])
                        for h in range(2):
                            nc.scalar.dma_start(
                                ktc_d[h][m * P:(m + 1) * P, :],
                                st[:, h * HR:(h + 1) * HR])
                    for h in range(2):
                        nc.gpsimd.collective_compute(
                            "AllGather", mybir.AluOpType.bypass, replica_groups=RG,
                            ins=[ktc_d[h].ap().opt()], outs=[ag_k[h].ap().opt()])

                    # --- V_c = X_c @ Wv.T + bv ---
                    v_ps = [pspool.tile([P, R], F32, name=f"vps{i}", tag="ps")
                            for i in range(8)]
                    for k in range(DC):
                        wv_t = wpool.tile([P, D], F16, name="wv_t", tag="w")
                        nc.sync.dma_start(wv_t, wvt[k * P:(k + 1) * P, :])
                        for rt in range(4):
                            for db in range(2):
                                nc.tensor.matmul(
                                    v_ps[rt * 2 + db],
                                    xt_t[k][:, rt * P:(rt + 1) * P],
                                    wv_t[:, db * 512:(db + 1) * 512],
                                    start=(k == 0), stop=(k == DC - 1))
                    for rt in range(4):
                        for db in range(2):
                            st = stpool.tile([P, R], F16, name="st_v", tag="st")
                            nc.vector.tensor_add(
                                st, v_ps[rt * 2 + db], bvb_t[:, db * 512:(db + 1) * 512])
                            nc.scalar.dma_start(
                                vc_d[rt // 2][(rt % 2) * P:(rt % 2 + 1) * P,
                                              db * 512:(db + 1) * 512], st)
                    for h in range(2):
                        nc.gpsimd.collective_compute(
                            "AllGather", mybir.AluOpType.bypass, replica_groups=RG,
                            ins=[vc_d[h].ap().opt()], outs=[ag_v[h].ap().opt()])

                    # --- Q_c.T = Wq @ X_c.T + bq ---
                    qt_t = [qpool.tile([P, R], F16, name=f"qt{m}") for m in range(DC)]
                    q_ps = [pspool.tile([P, R], F32, name=f"qps{m}", tag="ps")
                            for m in range(DC)]
                    for k in range(DC):
                        wq_t = wpool.tile([P, D], F16, name="wq_t", tag="w")
                        nc.sync.dma_start(wq_t, wqt[k * P:(k + 1) * P, :])
                        for m in range(DC):
                            nc.tensor.matmul(
                                q_ps[m], wq_t[:, m * P:(m + 1) * P], xt_t[k],
                                start=(k == 0), stop=(k == DC - 1))
                    for m in range(DC):
                        nc.vector.tensor_scalar_add(qt_t[m], q_ps[m], bq_t[:, m:m + 1])

                # ---- gap fillers while AG(K0) completes ----
                # x1_q.T[m] = Wl[:, D:].T-chunks @ Q.T (+ bl folded in)
                x1q_t = [qfpool.tile([P, R], F32, name=f"x1q{m}") for m in range(DC)]
                x1q_ps = [pspool.tile([P, R], F32, name=f"x1qps{m}", tag="ps")
                          for m in range(DC)]
                for k in range(DC):
                    wl_t = wpool.tile([P, D], F16, name="wl_t", tag="w")
                    nc.sync.dma_start(wl_t, wlt[(DC + k) * P:(DC + k + 1) * P, :])
                    for m in range(DC):
                        nc.tensor.matmul(
                            x1q_ps[m], wl_t[:, m * P:(m + 1) * P], qt_t[k],
                            start=(k == 0), stop=(k == DC - 1))
                for m in range(DC):
                    nc.vector.tensor_scalar_add(x1q_t[m], x1q_ps[m], bl_t[:, m:m + 1])

                # tail HK k-chunks of h-g0's Q part (+ ba folded in)
                hq0_t = [qfpool.tile([P, R], F32, name=f"hq0_{m}") for m in range(DC)]
                hq0_ps = [pspool.tile([P, R], F32, name=f"hq0ps{m}", tag="ps")
                          for m in range(DC)]
                for j in range(HK):
                    k = TDC - HK + j
                    wa_t = wpool.tile([P, D], F16, name="wa_t", tag="w")
                    nc.sync.dma_start(wa_t, wat[k * P:(k + 1) * P, 0:D])
                    for m in range(DC):
                        nc.tensor.matmul(
                            hq0_ps[m], wa_t[:, m * P:(m + 1) * P], qt_t[k - DC],
                            start=(j == 0), stop=(j == HK - 1))
                for m in range(DC):
                    nc.vector.tensor_scalar_add(hq0_t[m], hq0_ps[m], ba_t[:, m:m + 1])

                # ============ scoresT + exp + sums ============
                with tc.tile_pool(name="epool", bufs=1) as epool:
                    exp_t = [epool.tile([P, R], F16, name=f"exp{i}")
                             for i in range(NK)]
                    sums_ps = pspool.tile([1, R], F32, name="sums_ps", tag="ps")

                    def sums_mm(i):
                        nc.tensor.matmul(
                            sums_ps, ones_t, exp_t[i],
                            start=(i == 0), stop=(i == NK - 1),
                            skip_group_check=True)

                    for i, (rr, ss) in enumerate(KT_ORDER):
                        h, sh = (0, ss) if ss < 2 else (1, ss - 2)
                        kl = wpool.tile([P, D], F16, name="kl", tag="w")
                        eng = nc.sync if i % 2 == 0 else nc.scalar
                        eng.dma_start(
                            kl.rearrange("p (c n) -> p c n", c=DC),
                            ag_k[h][rr * D:(rr + 1) * D, sh * P:(sh + 1) * P]
                            .rearrange("(c p) n -> p c n", p=P))
                        sc_ps = pspool.tile([P, R], F32, name="sc_ps", tag="ps")
                        for c in range(DC):
                            nc.tensor.matmul(
                                sc_ps, kl[:, c * P:(c + 1) * P], qt_t[c],
                                start=(c == 0), stop=(c == DC - 1))
                        nc.scalar.activation(
                            exp_t[i], sc_ps, mybir.ActivationFunctionType.Exp,
                            bias=0.0, scale=1.0 / 32.0)
                        if i > 0:
                            sums_mm(i - 1)    # one behind: exp(i-1) surely done
                    sums_mm(NK - 1)

                    # reciprocal + broadcast to all partitions
                    recip_t = cpool.tile([1, R], F32, name="recip_t")
                    nc.vector.reciprocal(recip_t, sums_ps)
                    bc_ps = pspool.tile([P, R], F32, name="bc_ps", tag="ps")
                    nc.tensor.matmul(bc_ps, ones_row, recip_t, start=True, stop=True)
                    bc_t = cpool.tile([P, R], F32, name="bc_t")
                    nc.vector.tensor_copy(bc_t, bc_ps)

                    # ============ V_T = (alpha @ V).T ============
                    vt_ps = [pspool.tile([P, R], F32, name=f"vtps{m}", tag="ps")
                             for m in range(DC)]
                    for i, (rr, ss) in enumerate(KT_ORDER):
                        h, sh = (0, ss) if ss < 2 else (1, ss - 2)
                        vl = vlpool.tile([P, D], F16, name="vl", tag="vl")
                        eng = nc.sync if i % 2 == 0 else nc.scalar
                        eng.dma_start(
                            vl, ag_v[h][rr * HR + sh * P:rr * HR + (sh + 1) * P, :])
                        for m in range(DC):
                            nc.tensor.matmul(
                                vt_ps[m], vl[:, m * P:(m + 1) * P], exp_t[i],
                                start=(i == 0), stop=(i == NK - 1),
                                skip_group_check=True)
                    vt_t = [vtpool.tile([P, R], F16, name=f"vt{m}")
                            for m in range(DC)]
                    for m in range(DC):
                        nc.vector.tensor_mul(vt_t[m], vt_ps[m], bc_t)

                # ============ x1 (V-half), h, GLU ============
                def xk(k):
                    return vt_t[k] if k < DC else qt_t[k - DC]

                with tc.tile_pool(name="fpool", bufs=1) as fpool, \
                     tc.tile_pool(name="wg1pool", bufs=1) as wg1pool:
                    x1_ps = [pspool.tile([P, R], F32, name=f"x1ps{m}", tag="ps")
                             for m in range(DC)]
                    for k in range(DC):
                        wl_t = wpool.tile([P, D], F16, name="wl_t", tag="w")
                        nc.sync.dma_start(wl_t, wlt[k * P:(k + 1) * P, :])
                        for m in range(DC):
                            nc.tensor.matmul(
                                x1_ps[m], wl_t[:, m * P:(m + 1) * P], vt_t[k],
                                start=(k == 0), stop=(k == DC - 1))
                    x1_t = [fpool.tile([P, R], F32, name=f"x1{m}") for m in range(DC)]
                    for m in range(DC):
                        nc.vector.tensor_add(x1_t[m], x1_ps[m], x1q_t[m])

                    # h group 0 (a part): k-chunks [0, TDC-HK), Q-tail was
                    # precomputed into hq0_t
                    a_t = [fpool.tile([P, R], F32, name=f"a{m}") for m in range(DC)]
                    h_ps = [pspool.tile([P, R], F32, name=f"hps0_{m}", tag="ps")
                            for m in range(DC)]
                    for k in range(TDC - HK):
                        wa_t = wpool.tile([P, D], F16, name="wa_t", tag="w")
                        nc.sync.dma_start(wa_t, wat[k * P:(k + 1) * P, 0:D])
                        for m in range(DC):
                            nc.tensor.matmul(
                                h_ps[m], wa_t[:, m * P:(m + 1) * P], xk(k),
                                start=(k == 0), stop=(k == TDC - HK - 1))
                    for m in range(DC):
                        nc.vector.tensor_add(a_t[m], h_ps[m], hq0_t[m])

                    # h group 1 (b part): preload all 16 wa tiles, loop
                    # m-outer so each output column block finishes early and
                    # the GLU/output tail overlaps remaining matmuls.
                    wg1_t = [wg1pool.tile([P, D], F16, name=f"wg1_{k}")
                             for k in range(TDC)]
                    for k in range(TDC):
                        eng = nc.sync if k % 2 == 0 else nc.scalar
                        eng.dma_start(wg1_t[k], wat[k * P:(k + 1) * P, D:TD])
                    for m in range(DC):
                        hg1 = pspool.tile([P, R], F32, name=f"hps1_{m}", tag="ps")
                        for k in range(TDC):
                            nc.tensor.matmul(
                                hg1, wg1_t[k][:, m * P:(m + 1) * P], xk(k),
                                start=(k == 0), stop=(k == TDC - 1))
                        sig = fpool.tile([P, R], F32, name="sig", tag="sig", bufs=2)
                        nc.scalar.activation(
                            sig, hg1, mybir.ActivationFunctionType.Sigmoid,
                            bias=ba_t[:, DC + m:DC + m + 1], scale=1.0)
                        nc.vector.tensor_mul(a_t[m], a_t[m], sig)
                        nc.vector.tensor_mul(a_t[m], x1_t[m], a_t[m])
                        nc.scalar.dma_start(out[m * P:(m + 1) * P, :], a_t[m])

    nc.compile()
    return nc


_NC = None


def _get_nc():
    global _NC
    if _NC is None:
        _NC = build_nc()
    return _NC


def make_in_maps(input_features, Wq, bq, Wk, bk, Wv, bv, Wl, bl, Wa, ba):
    f = np.ascontiguousarray
    x = np.asarray(input_features, dtype=np.float32)
    xt_full = f(x.T.astype(np.float16))                  # [D, N]
    wqt = f(np.asarray(Wq, np.float32).T.astype(np.float16))
    wkt = f(np.asarray(Wk, np.float32).T.astype(np.float16))
    wvt = f(np.asarray(Wv, np.float32).T.astype(np.float16))
    wlt = f(np.asarray(Wl, np.float32).T.astype(np.float16))   # [2D, D]
    wat = f(np.asarray(Wa, np.float32).T.astype(np.float16))   # [2D, 2D]
    bq_r = f(np.asarray(bq, np.float32).reshape(DC, P).T)      # [P, DC]
    bk_r = f(np.asarray(bk, np.float32).reshape(DC, P).T)
    bl_r = f(np.asarray(bl, np.float32).reshape(DC, P).T)
    ba_r = f(np.asarray(ba, np.float32).reshape(TDC, P).T)     # [P, TDC]
    bvb = f(np.broadcast_to(np.asarray(bv, np.float32), (P, D)))
    in_maps = []
    for c in range(NCORES):
        in_maps.append({
            "xt": f(xt_full[:, c * R:(c + 1) * R]),
            "wqt": wqt, "wkt": wkt, "wvt": wvt, "wlt": wlt, "wat": wat,
            "bq": bq_r, "bk": bk_r, "bvb": bvb, "bl": bl_r, "ba": ba_r,
        })
    return in_maps


def run(in_maps, trace=False):
    nc = _get_nc()
    return bass_utils.run_bass_kernel_spmd(
        nc, in_maps, core_ids=list(range(NCORES)), trace=trace)


def kernel(input_features, Wq, bq, Wk, bk, Wv, bv, Wl, bl, Wa, ba):
    in_maps = make_in_maps(input_features, Wq, bq, Wk, bk, Wv, bv, Wl, bl, Wa, ba)
    res = run(in_maps)
    out = np.empty((N, D), dtype=np.float32)
    for c in range(NCORES):
        out[c * R:(c + 1) * R, :] = res.results[c]["out"].T
    return out
